# revision 40
# baseline (speedup 1.0000x reference)
"""DropBlock kernel for Trainium2, 8 NeuronCores, batch-sharded data parallel.

Reference computation (B,C,H,W = 128,64,56,56, block=5, gamma=0.02):
    mask    = (noise < gamma)                       # (B,C,52,52) corner drops
    dilated = maxpool5x5_full_pad(mask)             # (B,C,56,56)
    block_mask = 1 - dilated
    out = block_mask * x * (numel / sum(block_mask))

Kernel formulation (exact, no mask materialization in f32):
    d = noise - gamma_lo  (f32 subtract; sign/zero exact by Sterbenz, then
                           bf16 cast which preserves sign, never rounds to 0)
    block_mask[h,w] = ( min_{5x5 window}(d) > 0 )   # min-pool == dilated drop
    count = sum(block_mask) via fused accum, AllReduce across 8 cores.

Each core: 16 batches x 64 ch = 1024 images -> 8 tiles of 128 images
(images on partitions, image pixels along the free dimension).  The 5-wide
separable min uses log-step shifts (3 tensor_tensor ops per axis) on
1.0-padded buffers so no boundary special cases are needed.
"""

import sys

sys.path.insert(0, "/opt/trn_rl_repo")

import numpy as np

import concourse.bacc as bacc
import concourse.bass as bass
import concourse.tile as tile
import concourse.mybir as mybir
from concourse import bass_isa
from concourse.bass_utils import run_bass_kernel_spmd

N_CORES = 8
B, C, H, W = 128, 64, 56, 56
BLK = 5
GAMMA = 0.02
NH, NW = H - (BLK - 1), W - (BLK - 1)  # 52, 52 noise dims
B_SH = B // N_CORES  # 16 batches per core
IMGS = B_SH * C  # 1024 images per core
P = 128  # partitions
NTILES = IMGS // P  # 8 tiles per core
NPIX = NH * NW  # 2704 noise pixels/image
OPIX = H * W  # 3136 out pixels/image
TROWS = NH + 2 * (BLK - 1)  # 60 rows in padded vertical buffer
TFLAT = TROWS * NW  # 3120
VPW = NW + 2 * (BLK - 1)  # 60 cols in padded horizontal buffer (4+52+4)
COUNT_M = float(B * C * H * W)  # 25690112.0

# Largest f32 strictly below 0.02f: keep <=> noise >= 0.02f <=> noise-g' > 0,
# so the mask is Relu(Sign(min-pool(noise-g'))) with exact {0,1} handling.
GAMMA_LO = float(np.nextafter(np.float32(GAMMA), np.float32(0)))

F32 = mybir.dt.float32
BF16 = mybir.dt.bfloat16
MIN = mybir.AluOpType.min
MULT = mybir.AluOpType.mult
FP8 = mybir.dt.float8e4

X_PREFETCH = 8

_CACHE = {}


def _build(single_core=False, repeat=1, no_cc=False):
    """Build + compile the SPMD bass module once.

    single_core=True builds a collective-free variant (the per-core count is
    used directly as the global count) for cost-model simulation only.
    repeat>1 unrolls the whole pipeline k times (benchmarking only).
    no_cc=True skips the AllReduce on the 8-core build (timing probe only —
    results are wrong by the per-core/global count ratio).
    """
    nc = bacc.Bacc("TRN2", target_bir_lowering=False, debug=False,
                   num_devices=1 if single_core else N_CORES)
    noise_ap = nc.dram_tensor("noise", [IMGS, NPIX], F32,
                              kind="ExternalInput").ap()
    x_ap = nc.dram_tensor("x", [IMGS, OPIX], F32, kind="ExternalInput").ap()
    out_ap = nc.dram_tensor("out", [IMGS, OPIX], F32,
                            kind="ExternalOutput").ap()

    with tile.TileContext(nc) as tc:
        with (
            tc.tile_pool(name="nraw", bufs=2) as nraw_pool,
            tc.tile_pool(name="work", bufs=5) as work_pool,
            tc.tile_pool(name="vp", bufs=2) as vp_pool,
            tc.tile_pool(name="dmin", bufs=2) as dmin_pool,
            tc.tile_pool(name="mask", bufs=1) as mask_pool,
            tc.tile_pool(name="stats", bufs=1) as stats_pool,
            tc.tile_pool(name="xio", bufs=X_PREFETCH) as x_pool,
            tc.tile_pool(name="dram", bufs=1, space="DRAM") as dram_pool,
        ):
            mask_store = mask_pool.tile([P, NTILES * OPIX], FP8)
            partials = stats_pool.tile([P, NTILES], F32)
            gbias = stats_pool.tile([P, 1], F32)
            nc.vector.memset(gbias[:], -GAMMA_LO)

            for rep in range(repeat):
                _emit_once(nc, tc, noise_ap, x_ap, out_ap, mask_store,
                           partials, gbias, nraw_pool, work_pool,
                           vp_pool, dmin_pool, stats_pool, x_pool,
                           dram_pool, single_core or no_cc, rep)

    nc.compile()
    return nc


def _emit_once(nc, tc, noise_ap, x_ap, out_ap, mask_store, partials, gbias,
               nraw_pool, work_pool, vp_pool, dmin_pool, stats_pool,
               x_pool, dram_pool, single_core, rep):
    # ---------------- phase 1: block mask + counts ----------------
    xts = {}
    for t in range(NTILES):
        nraw = nraw_pool.tile([P, NPIX], F32, name=f"nraw{rep}_{t}",
                              tag="nraw")
        nc.sync.dma_start(nraw[:], noise_ap[bass.ts(t, P), :])
        if t < X_PREFETCH:
            # prefetch x tiles early on the same queue, behind this tile's
            # noise load; the sync queue stays ahead of the ~10us/tile
            # compute cadence and the scalar queue stays compute-only.
            xts[t] = x_pool.tile([P, OPIX], F32, name=f"xt{rep}_{t}",
                                 tag="xt")
            nc.sync.dma_start(xts[t][:], x_ap[bass.ts(t, P), :])

        # T: (60,52) bf16, rows 0..3 / 56..59 = 1.0 pad,
        # rows 4..55 = noise - gamma_lo
        tb = work_pool.tile([P, TFLAT], BF16, name=f"tb{rep}_{t}", tag="w")
        nc.gpsimd.memset(tb[:, 0:(BLK - 1) * NW], 1.0)
        nc.gpsimd.memset(tb[:, (NH + BLK - 1) * NW:TFLAT], 1.0)
        nc.scalar.activation(
            tb[:, (BLK - 1) * NW:(NH + BLK - 1) * NW], nraw[:],
            mybir.ActivationFunctionType.Identity, bias=gbias[:, 0:1])

        # vertical min pool, log-step: windows of 2, 4, then 5.
        # A rows 0..2 and 56..58 are mins of pad rows only (= 1.0): memset
        # them and run DVE only over the data-dependent rows 3..55.
        a = work_pool.tile([P, (TROWS - 1) * NW], BF16, name=f"a{rep}_{t}",
                        tag="w")  # 59 rows
        nc.gpsimd.memset(a[:, 0:3 * NW], 1.0)
        nc.gpsimd.memset(a[:, 56 * NW:(TROWS - 1) * NW], 1.0)
        nc.vector.tensor_tensor(
            a[:, 3 * NW:56 * NW], tb[:, 3 * NW:56 * NW],
            tb[:, 4 * NW:57 * NW], MIN)
        # B row 0 = min(A0, A2) = 1.0 likewise.
        bt = work_pool.tile([P, (TROWS - 3) * NW], BF16, name=f"bt{rep}_{t}",
                         tag="w")  # 57 rows
        nc.gpsimd.memset(bt[:, 0:NW], 1.0)
        nc.vector.tensor_tensor(
            bt[:, NW:(TROWS - 3) * NW], a[:, NW:(TROWS - 3) * NW],
            a[:, 3 * NW:(TROWS - 1) * NW], MIN)
        # V[r] = min(B[r], T[r+4]), r in 0..55 -> into padded Vp
        vp = vp_pool.tile([P, H * VPW], BF16, name=f"vp{rep}_{t}", tag="vp")
        vp3 = vp[:].rearrange("p (h w) -> p h w", w=VPW)
        nc.gpsimd.memset(vp3[:, :, 0:BLK - 1], 1.0)
        nc.gpsimd.memset(vp3[:, :, W:VPW], 1.0)
        bt3 = bt[:].rearrange("p (h w) -> p h w", w=NW)
        tb3 = tb[:].rearrange("p (h w) -> p h w", w=NW)
        nc.vector.tensor_tensor(
            vp3[:, :, BLK - 1:BLK - 1 + NW], bt3[:, 0:H, :],
            tb3[:, BLK - 1:TROWS, :], MIN)

        # horizontal min pool, log-step (flat shifted APs; the out-of-row
        # tail elements are junk but never read)
        HV = H * VPW
        a2 = work_pool.tile([P, HV], BF16, name=f"a2{rep}_{t}", tag="w")
        nc.vector.tensor_tensor(
            a2[:, 0:HV - 1], vp[:, 0:HV - 1], vp[:, 1:HV], MIN)
        b2 = work_pool.tile([P, HV], BF16, name=f"b2{rep}_{t}", tag="w")
        nc.vector.tensor_tensor(
            b2[:, 0:HV - 2], a2[:, 0:HV - 2], a2[:, 2:HV], MIN)
        b23 = b2[:].rearrange("p (h w) -> p h w", w=VPW)
        dm = dmin_pool.tile([P, OPIX], BF16, name=f"dm{rep}_{t}", tag="dm")
        dm3 = dm[:].rearrange("p (h w) -> p h w", w=W)
        nc.vector.tensor_tensor(
            dm3[:, :, :], b23[:, :, 0:W], vp3[:, :, BLK - 1:VPW], MIN)

        # block_mask = (dmin > 0); count per partition into partials[:, t].
        # Tiles 0..n-2: ACT Relu(Sign(d)) with fused f32 accum, overlapped
        # with later tiles' DVE work.  Last tile: DVE is_gt + f32 reduce so
        # the count (-> collective -> scale) skips the ACT queue.  (The DVE
        # tensor_scalar accum_out accumulates in the fp8 OUT dtype and
        # saturates, so the reduce is a separate exact op.)
        mslice = mask_store[:, t * OPIX:(t + 1) * OPIX]
        if t == NTILES - 1:
            nc.vector.tensor_scalar(mslice, dm[:], 0.0, 1.0,
                                    mybir.AluOpType.is_gt, MULT)
            nc.vector.tensor_reduce(partials[:, t:t + 1], mslice,
                                    mybir.AxisListType.X,
                                    mybir.AluOpType.add)
        else:
            nc.scalar.activation(dm[:], dm[:],
                                 mybir.ActivationFunctionType.Sign)
            nc.scalar.activation(
                mslice, dm[:], mybir.ActivationFunctionType.Relu,
                accum_out=partials[:, t:t + 1])

    # ------------- global count -> scale = M / count_ones -------------
    ptot = stats_pool.tile([P, 1], F32, name=f"ptot{rep}", tag="ptot")
    nc.vector.tensor_reduce(ptot[:], partials[:], mybir.AxisListType.X,
                            mybir.AluOpType.add)
    pall = stats_pool.tile([P, 1], F32, name=f"pall{rep}", tag="pall")
    nc.gpsimd.partition_all_reduce(pall[:], ptot[:], channels=P,
                                   reduce_op=bass_isa.ReduceOp.add)
    if single_core:
        tot_sb = pall
    else:
        cc_in = dram_pool.tile([P, 1], F32, name=f"cc_in{rep}", tag="cc_in")
        cc_out = dram_pool.tile([P, 1], F32, name=f"cc_out{rep}",
                                tag="cc_out")
        nc.sync.dma_start(cc_in[:], pall[:])
        nc.gpsimd.collective_compute(
            "AllReduce", mybir.AluOpType.add,
            replica_groups=[list(range(N_CORES))],
            ins=[cc_in.opt()], outs=[cc_out.opt()])
        tot_sb = stats_pool.tile([P, 1], F32, name=f"tot{rep}", tag="tot")
        nc.sync.dma_start(tot_sb[:], cc_out[:])
    recip = stats_pool.tile([P, 1], F32, name=f"recip{rep}", tag="recip")
    nc.vector.reciprocal(recip[:], tot_sb[:])
    scale_sb = stats_pool.tile([P, 1], F32, name=f"scale{rep}", tag="scale")
    nc.vector.tensor_scalar_mul(scale_sb[:], recip[:], COUNT_M)

    # ---------------- phase 2: out = (x*scale)*mask ----------------
    for t in range(NTILES):
        if t in xts:
            xt = xts[t]
        else:
            xt = x_pool.tile([P, OPIX], F32, name=f"xt{rep}_{t}", tag="xt")
            nc.scalar.dma_start(xt[:], x_ap[bass.ts(t, P), :])
        nc.vector.scalar_tensor_tensor(
            xt[:], xt[:], scale_sb[:, 0:1],
            mask_store[:, t * OPIX:(t + 1) * OPIX], MULT, MULT)
        # alternate stores across both HWDGE queues (scalar queue is idle
        # in phase 2) so the final drain is paced by aggregate DMA BW, not
        # one queue's serialization
        eng = nc.sync if t % 2 == 0 else nc.scalar
        eng.dma_start(out_ap[bass.ts(t, P), :], xt[:])


def _get_nc():
    if "nc" not in _CACHE:
        _CACHE["nc"] = _build()
    return _CACHE["nc"]


def kernel(x: np.ndarray, noise: np.ndarray) -> np.ndarray:
    x = np.asarray(x, dtype=np.float32)
    noise = np.asarray(noise, dtype=np.float32)
    assert x.shape == (B, C, H, W) and noise.shape == (B, C, NH, NW)
    nc = _get_nc()
    in_maps = []
    for i in range(N_CORES):
        xs = np.ascontiguousarray(x[i * B_SH:(i + 1) * B_SH]).reshape(
            IMGS, OPIX)
        ns = np.ascontiguousarray(noise[i * B_SH:(i + 1) * B_SH]).reshape(
            IMGS, NPIX)
        in_maps.append({"x": xs, "noise": ns})
    res = run_bass_kernel_spmd(nc, in_maps, list(range(N_CORES)))
    out = np.empty((B, C, H, W), dtype=np.float32)
    for i in range(N_CORES):
        out[i * B_SH:(i + 1) * B_SH] = res.results[i]["out"].reshape(
            B_SH, C, H, W)
    return out


# revision 41
# speedup vs baseline: 1.0016x; 1.0016x over previous
"""DropBlock kernel for Trainium2, 8 NeuronCores, batch-sharded data parallel.

Reference computation (B,C,H,W = 128,64,56,56, block=5, gamma=0.02):
    mask    = (noise < gamma)                       # (B,C,52,52) corner drops
    dilated = maxpool5x5_full_pad(mask)             # (B,C,56,56)
    block_mask = 1 - dilated
    out = block_mask * x * (numel / sum(block_mask))

Kernel formulation (exact, no mask materialization in f32):
    d = noise - gamma_lo  (f32 subtract; sign/zero exact by Sterbenz, then
                           bf16 cast which preserves sign, never rounds to 0)
    block_mask[h,w] = ( min_{5x5 window}(d) > 0 )   # min-pool == dilated drop
    count = sum(block_mask) via fused accum, AllReduce across 8 cores.

Each core: 16 batches x 64 ch = 1024 images -> 8 tiles of 128 images
(images on partitions, image pixels along the free dimension).  The 5-wide
separable min uses log-step shifts (3 tensor_tensor ops per axis) on
1.0-padded buffers so no boundary special cases are needed.
"""

import sys

sys.path.insert(0, "/opt/trn_rl_repo")

import numpy as np

import concourse.bacc as bacc
import concourse.bass as bass
import concourse.tile as tile
import concourse.mybir as mybir
from concourse import bass_isa
from concourse.bass_utils import run_bass_kernel_spmd

N_CORES = 8
B, C, H, W = 128, 64, 56, 56
BLK = 5
GAMMA = 0.02
NH, NW = H - (BLK - 1), W - (BLK - 1)  # 52, 52 noise dims
B_SH = B // N_CORES  # 16 batches per core
IMGS = B_SH * C  # 1024 images per core
P = 128  # partitions
NTILES = IMGS // P  # 8 tiles per core
NPIX = NH * NW  # 2704 noise pixels/image
OPIX = H * W  # 3136 out pixels/image
TROWS = NH + 2 * (BLK - 1)  # 60 rows in padded vertical buffer
TFLAT = TROWS * NW  # 3120
VPW = NW + 2 * (BLK - 1)  # 60 cols in padded horizontal buffer (4+52+4)
COUNT_M = float(B * C * H * W)  # 25690112.0

# Largest f32 strictly below 0.02f: keep <=> noise >= 0.02f <=> noise-g' > 0,
# so the mask is Relu(Sign(min-pool(noise-g'))) with exact {0,1} handling.
GAMMA_LO = float(np.nextafter(np.float32(GAMMA), np.float32(0)))

F32 = mybir.dt.float32
BF16 = mybir.dt.bfloat16
MIN = mybir.AluOpType.min
MULT = mybir.AluOpType.mult
FP8 = mybir.dt.float8e4

X_PREFETCH = 8

_CACHE = {}


def _build(single_core=False, repeat=1, no_cc=False):
    """Build + compile the SPMD bass module once.

    single_core=True builds a collective-free variant (the per-core count is
    used directly as the global count) for cost-model simulation only.
    repeat>1 unrolls the whole pipeline k times (benchmarking only).
    no_cc=True skips the AllReduce on the 8-core build (timing probe only —
    results are wrong by the per-core/global count ratio).
    """
    nc = bacc.Bacc("TRN2", target_bir_lowering=False, debug=False,
                   num_devices=1 if single_core else N_CORES)
    noise_ap = nc.dram_tensor("noise", [IMGS, NPIX], F32,
                              kind="ExternalInput").ap()
    x_ap = nc.dram_tensor("x", [IMGS, OPIX], F32, kind="ExternalInput").ap()
    out_ap = nc.dram_tensor("out", [IMGS, OPIX], F32,
                            kind="ExternalOutput").ap()

    with tile.TileContext(nc) as tc:
        with (
            tc.tile_pool(name="nraw", bufs=2) as nraw_pool,
            tc.tile_pool(name="work", bufs=5) as work_pool,
            tc.tile_pool(name="vp", bufs=2) as vp_pool,
            tc.tile_pool(name="dmin", bufs=2) as dmin_pool,
            tc.tile_pool(name="mask", bufs=1) as mask_pool,
            tc.tile_pool(name="stats", bufs=1) as stats_pool,
            tc.tile_pool(name="xio", bufs=X_PREFETCH) as x_pool,
            tc.tile_pool(name="dram", bufs=1, space="DRAM") as dram_pool,
        ):
            mask_store = mask_pool.tile([P, NTILES * OPIX], FP8)
            partials = stats_pool.tile([P, NTILES], F32)
            gbias = stats_pool.tile([P, 1], F32)
            nc.vector.memset(gbias[:], -GAMMA_LO)

            for rep in range(repeat):
                _emit_once(nc, tc, noise_ap, x_ap, out_ap, mask_store,
                           partials, gbias, nraw_pool, work_pool,
                           vp_pool, dmin_pool, stats_pool, x_pool,
                           dram_pool, single_core or no_cc, rep)

    nc.compile()
    return nc


def _emit_once(nc, tc, noise_ap, x_ap, out_ap, mask_store, partials, gbias,
               nraw_pool, work_pool, vp_pool, dmin_pool, stats_pool,
               x_pool, dram_pool, single_core, rep):
    # ---------------- phase 1: block mask + counts ----------------
    xts = {}
    for t in range(NTILES):
        nraw = nraw_pool.tile([P, NPIX], F32, name=f"nraw{rep}_{t}",
                              tag="nraw")
        nc.sync.dma_start(nraw[:], noise_ap[bass.ts(t, P), :])
        if t < X_PREFETCH:
            # prefetch x tiles early on the same queue, behind this tile's
            # noise load; the sync queue stays ahead of the ~10us/tile
            # compute cadence and the scalar queue stays compute-only.
            xts[t] = x_pool.tile([P, OPIX], F32, name=f"xt{rep}_{t}",
                                 tag="xt")
            nc.sync.dma_start(xts[t][:], x_ap[bass.ts(t, P), :])

        # T: (60,52) bf16, rows 0..3 / 56..59 = 1.0 pad,
        # rows 4..55 = noise - gamma_lo
        tb = work_pool.tile([P, TFLAT], BF16, name=f"tb{rep}_{t}", tag="w")
        nc.gpsimd.memset(tb[:, 0:(BLK - 1) * NW], 1.0)
        nc.gpsimd.memset(tb[:, (NH + BLK - 1) * NW:TFLAT], 1.0)
        nc.scalar.activation(
            tb[:, (BLK - 1) * NW:(NH + BLK - 1) * NW], nraw[:],
            mybir.ActivationFunctionType.Identity, bias=gbias[:, 0:1])

        # vertical min pool, log-step: windows of 2, 4, then 5.
        # A rows 0..2 and 56..58 are mins of pad rows only (= 1.0): memset
        # them and run DVE only over the data-dependent rows 3..55.
        a = work_pool.tile([P, (TROWS - 1) * NW], BF16, name=f"a{rep}_{t}",
                        tag="w")  # 59 rows
        nc.gpsimd.memset(a[:, 0:3 * NW], 1.0)
        nc.gpsimd.memset(a[:, 56 * NW:(TROWS - 1) * NW], 1.0)
        nc.vector.tensor_tensor(
            a[:, 3 * NW:56 * NW], tb[:, 3 * NW:56 * NW],
            tb[:, 4 * NW:57 * NW], MIN)
        # B row 0 = min(A0, A2) = 1.0 likewise.
        bt = work_pool.tile([P, (TROWS - 3) * NW], BF16, name=f"bt{rep}_{t}",
                         tag="w")  # 57 rows
        nc.gpsimd.memset(bt[:, 0:NW], 1.0)
        nc.vector.tensor_tensor(
            bt[:, NW:(TROWS - 3) * NW], a[:, NW:(TROWS - 3) * NW],
            a[:, 3 * NW:(TROWS - 1) * NW], MIN)
        # V[r] = min(B[r], T[r+4]), r in 0..55 -> into padded Vp
        vp = vp_pool.tile([P, H * VPW], BF16, name=f"vp{rep}_{t}", tag="vp")
        vp3 = vp[:].rearrange("p (h w) -> p h w", w=VPW)
        nc.gpsimd.memset(vp3[:, :, 0:BLK - 1], 1.0)
        nc.gpsimd.memset(vp3[:, :, W:VPW], 1.0)
        bt3 = bt[:].rearrange("p (h w) -> p h w", w=NW)
        tb3 = tb[:].rearrange("p (h w) -> p h w", w=NW)
        nc.vector.tensor_tensor(
            vp3[:, :, BLK - 1:BLK - 1 + NW], bt3[:, 0:H, :],
            tb3[:, BLK - 1:TROWS, :], MIN)

        # horizontal min pool, log-step (flat shifted APs; the out-of-row
        # tail elements are junk but never read)
        HV = H * VPW
        a2 = work_pool.tile([P, HV], BF16, name=f"a2{rep}_{t}", tag="w")
        nc.vector.tensor_tensor(
            a2[:, 0:HV - 1], vp[:, 0:HV - 1], vp[:, 1:HV], MIN)
        b2 = work_pool.tile([P, HV], BF16, name=f"b2{rep}_{t}", tag="w")
        nc.vector.tensor_tensor(
            b2[:, 0:HV - 2], a2[:, 0:HV - 2], a2[:, 2:HV], MIN)
        b23 = b2[:].rearrange("p (h w) -> p h w", w=VPW)
        dm = dmin_pool.tile([P, OPIX], BF16, name=f"dm{rep}_{t}", tag="dm")
        dm3 = dm[:].rearrange("p (h w) -> p h w", w=W)
        nc.vector.tensor_tensor(
            dm3[:, :, :], b23[:, :, 0:W], vp3[:, :, BLK - 1:VPW], MIN)

        # block_mask = (dmin > 0); count per partition into partials[:, t].
        # Tiles 0..n-2: ACT Relu(Sign(d)) with fused f32 accum, overlapped
        # with later tiles' DVE work.  Last tile: DVE is_gt + f32 reduce so
        # the count (-> collective -> scale) skips the ACT queue.  (The DVE
        # tensor_scalar accum_out accumulates in the fp8 OUT dtype and
        # saturates, so the reduce is a separate exact op.)
        mslice = mask_store[:, t * OPIX:(t + 1) * OPIX]
        if t == NTILES - 1:
            nc.vector.tensor_scalar(mslice, dm[:], 0.0, 1.0,
                                    mybir.AluOpType.is_gt, MULT)
            nc.vector.tensor_reduce(partials[:, t:t + 1], mslice,
                                    mybir.AxisListType.X,
                                    mybir.AluOpType.add)
        else:
            nc.scalar.activation(dm[:], dm[:],
                                 mybir.ActivationFunctionType.Sign)
            nc.scalar.activation(
                mslice, dm[:], mybir.ActivationFunctionType.Relu,
                accum_out=partials[:, t:t + 1])

    # ------------- global count -> scale = M / count_ones -------------
    # partials[0:7] are reduced while tile 7 is still computing; only the
    # tiny add of tile 7's count sits on the critical chain.
    phead = stats_pool.tile([P, 1], F32, name=f"phead{rep}", tag="phead")
    nc.vector.tensor_reduce(phead[:], partials[:, 0:NTILES - 1],
                            mybir.AxisListType.X, mybir.AluOpType.add)
    ptot = stats_pool.tile([P, 1], F32, name=f"ptot{rep}", tag="ptot")
    nc.vector.tensor_tensor(ptot[:], phead[:],
                            partials[:, NTILES - 1:NTILES],
                            mybir.AluOpType.add)
    pall = stats_pool.tile([P, 1], F32, name=f"pall{rep}", tag="pall")
    nc.gpsimd.partition_all_reduce(pall[:], ptot[:], channels=P,
                                   reduce_op=bass_isa.ReduceOp.add)
    if single_core:
        tot_sb = pall
    else:
        cc_in = dram_pool.tile([P, 1], F32, name=f"cc_in{rep}", tag="cc_in")
        cc_out = dram_pool.tile([P, 1], F32, name=f"cc_out{rep}",
                                tag="cc_out")
        nc.sync.dma_start(cc_in[:], pall[:])
        nc.gpsimd.collective_compute(
            "AllReduce", mybir.AluOpType.add,
            replica_groups=[list(range(N_CORES))],
            ins=[cc_in.opt()], outs=[cc_out.opt()])
        tot_sb = stats_pool.tile([P, 1], F32, name=f"tot{rep}", tag="tot")
        nc.sync.dma_start(tot_sb[:], cc_out[:])
    recip = stats_pool.tile([P, 1], F32, name=f"recip{rep}", tag="recip")
    nc.vector.reciprocal(recip[:], tot_sb[:])
    scale_sb = stats_pool.tile([P, 1], F32, name=f"scale{rep}", tag="scale")
    nc.vector.tensor_scalar_mul(scale_sb[:], recip[:], COUNT_M)

    # ---------------- phase 2: out = (x*scale)*mask ----------------
    for t in range(NTILES):
        if t in xts:
            xt = xts[t]
        else:
            xt = x_pool.tile([P, OPIX], F32, name=f"xt{rep}_{t}", tag="xt")
            nc.scalar.dma_start(xt[:], x_ap[bass.ts(t, P), :])
        nc.vector.scalar_tensor_tensor(
            xt[:], xt[:], scale_sb[:, 0:1],
            mask_store[:, t * OPIX:(t + 1) * OPIX], MULT, MULT)
        # alternate stores across both HWDGE queues (scalar queue is idle
        # in phase 2) so the final drain is paced by aggregate DMA BW, not
        # one queue's serialization
        eng = nc.sync if t % 2 == 0 else nc.scalar
        eng.dma_start(out_ap[bass.ts(t, P), :], xt[:])


def _get_nc():
    if "nc" not in _CACHE:
        _CACHE["nc"] = _build()
    return _CACHE["nc"]


def kernel(x: np.ndarray, noise: np.ndarray) -> np.ndarray:
    x = np.asarray(x, dtype=np.float32)
    noise = np.asarray(noise, dtype=np.float32)
    assert x.shape == (B, C, H, W) and noise.shape == (B, C, NH, NW)
    nc = _get_nc()
    in_maps = []
    for i in range(N_CORES):
        xs = np.ascontiguousarray(x[i * B_SH:(i + 1) * B_SH]).reshape(
            IMGS, OPIX)
        ns = np.ascontiguousarray(noise[i * B_SH:(i + 1) * B_SH]).reshape(
            IMGS, NPIX)
        in_maps.append({"x": xs, "noise": ns})
    res = run_bass_kernel_spmd(nc, in_maps, list(range(N_CORES)))
    out = np.empty((B, C, H, W), dtype=np.float32)
    for i in range(N_CORES):
        out[i * B_SH:(i + 1) * B_SH] = res.results[i]["out"].reshape(
            B_SH, C, H, W)
    return out
